# revision 14
# baseline (speedup 1.0000x reference)
"""Causal GQA attention on 8 TRN2 NeuronCores.

Problem: q [4096, 4096] = [bs*seq, 32 heads * 128], k/v [4096, 1024] =
[bs*seq, 8 kv heads * 128], causal softmax(q k^T / sqrt(128)) v with GQA
(4 query heads per kv head). f32 in/out.

Sharding: 8 cores = 2 batches x 4 head-groups. Each core owns one batch
and 8 query heads / 2 kv heads -- fully local, no collectives. Q and K are
handed to each core pre-permuted to [head_dim, head, seq] (host-side layout
marshalling in the shard step) so the contraction dim is already on
partitions; V is packed host-side with a fused ones column ([V_j | 1],
bf16) as the PV matmul wants it.

The kernel is organized around the ScalarE exp bottleneck (~139k PSUM
columns of exp per core at ~1 elem/cycle/lane): the S^T columns for all
(head, key-block) pairs form one continuous stream packed into rotating
[128, 1536] f32 PSUM regions (3 banks each, 2 regions = 6 banks), so every
exp is a single wide ACTIVATE (N=1536) instead of many narrow ones --
per-instruction overhead (~400ns) amortizes 12x better. The causal mask is
applied AFTER exp by a DVE multiply of each diagonal 128x128 P^T subtile
with a 0/1 lower-triangle (bf16 2x mode), keeping the QK->exp path free of
VectorE and making softmax denominators exact (masked probs are 0).

Per-core pipeline over regions r (91 per core):
  QK(r+1) on PE (K_j-stationary, <=512-wide bank-aligned chunks)
  || exp(r) on ScalarE (one ACTIVATE, scale folded in)
  || triangle masks(r) + chain normalizes on DVE
  || PV chains whose diagonal block lives in region r-1 on PE:
       acc[q,0:129] = sum_j P^T_j[:, s-subtile] @ [V_j | 1]
     (P^T-stationary accumulation in 2 rotating 1-bank PSUM tiles).

Walrus sync-wait limits (1 slot on DMA descriptors and LDWEIGHTS): all
loads land upfront in fresh buffers, tiny PE warmup matmuls absorb the
DMA semaphores into PE's vector clock (injected just before first use for
late-loaded pieces), and the triangle mask lives in a raw pre-Tile
preamble so it is dependency-free.

No max-subtraction softmax: logits are ~N(0,1) after scale, exp stays in
range; diag-block garbage (upper triangle) is finite and zeroed post-exp.
"""

import numpy as np

P = 128          # partitions / head_dim / key block
SEQ = 2048       # per-core sequence length
H = 8            # query heads per core
KV = 2           # kv heads per core
D = 128          # head dim
NB = SEQ // P    # 16 key blocks (also query subtiles) per head
SCALE = float(D) ** -0.5

REG_W = 1536     # S^T stream region width (3 PSUM banks of f32)
HEAD_W = sum(SEQ - P * j for j in range(NB))          # 17408 cols per head
STREAM_W = H * HEAD_W                                  # 139264 per core
NR = (STREAM_W + REG_W - 1) // REG_W                   # 91 regions

_NC = None


def _block_base(h, j):
    """Stream position of the first column of (head h, key block j)."""
    return h * HEAD_W + SEQ * j - 64 * j * (j - 1)


def _stream_layout():
    """Build-time bookkeeping: region -> QK segments / diag subtiles, and
    (h, j, s) -> (region, offset) for PV chain stationary slices."""
    segs = [[] for _ in range(NR)]    # (h, j, q0, width, region_off)
    diags = [[] for _ in range(NR)]   # (h, j, region_off)
    for h in range(H):
        for j in range(NB):
            w = SEQ - P * j
            base = _block_base(h, j)
            r0, off0 = divmod(base, REG_W)
            diags[r0].append((h, j, off0))
            c = 0
            while c < w:
                r, off = divmod(base + c, REG_W)
                take = min(w - c, REG_W - off, 512 - (off % 512))
                segs[r].append((h, j, P * j + c, take, off))
                c += take
    return segs, diags


def _pt_slice_loc(h, j, s):
    """Region and offset of P^T_j[:, s-th 128-query subtile] for head h."""
    return divmod(_block_base(h, j) + P * (s - j), REG_W)


def _build_nc():
    import concourse.bass as bass
    import concourse.bacc as bacc
    import concourse.mybir as mybir
    import concourse.tile as tile
    from contextlib import ExitStack

    f32 = mybir.dt.float32
    bf16 = mybir.dt.bfloat16
    Exp = mybir.ActivationFunctionType.Exp

    segs, diags = _stream_layout()

    nc = bacc.Bacc()
    qT_ext = nc.declare_dram_parameter("qT", [P, H, SEQ], bf16, isOutput=False)
    kT_ext = nc.declare_dram_parameter("kT", [P, KV, SEQ], bf16, isOutput=False)
    v_ext = nc.declare_dram_parameter("vones", [SEQ, KV * (D + 1)], bf16,
                                      isOutput=False)
    o_ext = nc.declare_dram_parameter("out", [SEQ, H * D], f32, isOutput=True)

    vd = v_ext.rearrange("(i p) c -> p i c", p=P)
    od = o_ext.rearrange("(i p) c -> p i c", p=P)

    # 0/1 lower-triangle (keep where q_local - k_local >= 0), built f32 in a
    # raw pre-Tile preamble (gpsimd in-order + barrier => dependency-free).
    tri_f32 = nc.alloc_sbuf_tensor("tri_f32", [P, P], f32).ap()
    nc.gpsimd.affine_select(
        out=tri_f32,
        in_=nc.const_aps.tensor(1.0, (P, P)),
        compare_op=mybir.AluOpType.is_ge,
        fill=0.0,
        base=0,
        pattern=[[1, P]],
        channel_multiplier=-1,
    )
    nc.all_engine_barrier()

    with ExitStack() as ctx:
        tc = ctx.enter_context(tile.TileContext(nc))
        singles = ctx.enter_context(tc.tile_pool(name="singles", bufs=1))
        pt_pool = ctx.enter_context(tc.tile_pool(name="pt", bufs=18))
        ob_pool = ctx.enter_context(tc.tile_pool(name="ob", bufs=2))
        r_pool = ctx.enter_context(tc.tile_pool(name="r", bufs=8))
        ps_st = ctx.enter_context(tc.tile_pool(name="ps_st", bufs=2, space="PSUM"))
        ps_pv = ctx.enter_context(tc.tile_pool(name="ps_pv", bufs=2, space="PSUM"))

        # ---- upfront loads, each into a fresh buffer on a fresh queue ----
        # Head 0 / kv 0 pieces come first so compute starts early.
        kt = singles.tile([P, KV, SEQ], bf16)        # [d, kv, key]
        qt = singles.tile([P, H, SEQ], bf16)         # [d, head, query]
        vones = singles.tile([P, NB, KV, D + 1], bf16)  # [k, block, kv, d|1]
        nc.gpsimd.dma_start(out=kt[:, 0, 0:256], in_=kT_ext.ap()[:, 0, 0:256])
        nc.gpsimd.dma_start(out=qt[:, 0, 0:1536], in_=qT_ext.ap()[:, 0, 0:1536])
        nc.gpsimd.dma_start(out=kt[:, 0, 256:1024], in_=kT_ext.ap()[:, 0, 256:1024])
        nc.gpsimd.dma_start(out=qt[:, 0, 1536:], in_=qT_ext.ap()[:, 0, 1536:])
        nc.gpsimd.dma_start(out=kt[:, 0, 1024:], in_=kT_ext.ap()[:, 0, 1024:])
        nc.gpsimd.dma_start(out=vones[:, :, 0, :], in_=vd[:, :, 0:D + 1])
        nc.gpsimd.dma_start(out=qt[:, 1:2, :], in_=qT_ext.ap()[:, 1:2, :])
        nc.gpsimd.dma_start(out=qt[:, 2:3, :], in_=qT_ext.ap()[:, 2:3, :])
        nc.gpsimd.dma_start(out=qt[:, 3:4, :], in_=qT_ext.ap()[:, 3:4, :])
        nc.gpsimd.dma_start(out=kt[:, 1:2, :], in_=kT_ext.ap()[:, 1:2, :])
        nc.gpsimd.dma_start(out=vones[:, :, 1, :], in_=vd[:, :, D + 1:])
        for i in range(4, H):
            nc.gpsimd.dma_start(out=qt[:, i:i + 1, :], in_=qT_ext.ap()[:, i:i + 1, :])

        # bf16 copy of the triangle for 2x-mode DVE masking
        tri01 = singles.tile([P, P], bf16)
        nc.vector.tensor_copy(out=tri01, in_=tri_f32)

        # ---- PE warmups: absorb DMA/DVE semaphores into PE's clock so real
        # matmuls never carry a second wait. Outputs unread.
        def warm(ap):
            wm = ps_pv.tile([2, 2], f32, tag="pvacc", name="wm")
            nc.tensor.matmul(wm[:1, :1], lhsT=ap, rhs=ap, start=True, stop=True)

        # HAM pre-warm: ~3.5us of back-to-back dummy matmuls on the (already
        # initialized) triangle so the PE clock gate opens before the first
        # real QK burst instead of ~20us in. These run while the DMAs land.
        hamwm = ps_pv.tile([P, P], f32, tag="pvacc", name="hamwm")
        for _ in range(22):
            nc.tensor.matmul(hamwm, lhsT=tri_f32.bitcast(bf16)[:, 0:P],
                             rhs=tri_f32.bitcast(bf16)[:, 0:P],
                             start=True, stop=True)

        warm(kt[:, 0, 0:1])
        warm(kt[:, 0, 256:257])
        warm(kt[:, 0, 1024:1025])
        warm(qt[:, 0, 0:1])
        warm(qt[:, 0, 1536:1537])
        warm(vones[:, 0, 0, 0:1])
        warm(tri01[:, 0:1])
        # exp table load early, overlapping the remaining DMAs
        actwarm = singles.tile([P, P], bf16)
        nc.scalar.activation(out=actwarm, in_=tri_f32, func=Exp, scale=SCALE)

        warmed = {("q", 0), ("q", 0.5), ("k", 0), ("k", 0.5), ("v", 0)}

        def warm_for_head(h):
            kvh = h // (H // KV)
            if ("q", h) not in warmed:
                warmed.add(("q", h))
                warm(qt[:, h, 0:1])
            if ("k", kvh) not in warmed:
                warmed.add(("k", kvh))
                warm(kt[:, kvh, 0:1])
            if ("v", kvh) not in warmed:
                warmed.add(("v", kvh))
                warm(vones[:, 0, kvh, 0:1])

        # ---- pipelined region loop ----
        st_tiles = {}
        pt_tiles = {}
        o_sbs = {}

        def emit_qk(r):
            st = ps_st.tile([P, REG_W], f32, name="st")
            st_tiles[r] = st
            for (h, j, q0, w, off) in segs[r]:
                warm_for_head(h)
                kvh = h // (H // KV)
                nc.tensor.matmul(
                    st[:, off:off + w],
                    lhsT=kt[:, kvh, j * P:(j + 1) * P],
                    rhs=qt[:, h, q0:q0 + w],
                    start=True,
                    stop=True,
                )

        def emit_act(r):
            w = min(REG_W, STREAM_W - r * REG_W)
            pt = pt_pool.tile([P, REG_W], bf16, name="pt")
            pt_tiles[r] = pt
            nc.scalar.activation(
                out=pt[:, 0:w], in_=st_tiles.pop(r)[:, 0:w],
                func=Exp, scale=SCALE,
            )

        def emit_masks(r):
            pt = pt_tiles[r]
            for (h, j, off) in diags[r]:
                nc.vector.tensor_mul(
                    out=pt[:, off:off + P],
                    in0=pt[:, off:off + P],
                    in1=tri01,
                )

        def emit_chain(h, s):
            kvh = h // (H // KV)
            if s == 0:
                o_sbs[h] = ob_pool.tile([P, NB, D + 1], f32, name="o_raw")
            acc = ps_pv.tile([P, D + 1], f32, tag="pvacc", name="pvacc")
            for j in range(s + 1):
                rr, off = _pt_slice_loc(h, j, s)
                nc.tensor.matmul(
                    acc,
                    lhsT=pt_tiles[rr][:, off:off + P],
                    rhs=vones[:, j, kvh, :],
                    start=(j == 0),
                    stop=(j == s),
                )
            # One cheap DVE op frees the PSUM acc (pool WAR) quickly; the
            # recip + normalize run from SBUF in batches of 4 subtiles.
            o_raw = o_sbs[h]
            nc.vector.tensor_copy(out=o_raw[:, s, :], in_=acc)
            if s % 4 == 3:
                rcp = r_pool.tile([P, 4], f32, name="rcp")
                nc.vector.reciprocal(rcp, o_raw[:, s - 3:s + 1, D])
                for i in range(4):
                    si = s - 3 + i
                    nc.vector.tensor_scalar_mul(
                        o_raw[:, si, 0:D], o_raw[:, si, 0:D], rcp[:, i:i + 1]
                    )
                nc.sync.dma_start(
                    out=od[:, s - 3:s + 1, h * D:(h + 1) * D],
                    in_=o_raw[:, s - 3:s + 1, 0:D],
                )

        # Iteration r: ACT(r) [needs QK(r), emitted 2 iters ago]; chains for
        # diag region r-1 (runnable: only need ACT(r-1)+mask(r-1)) BEFORE
        # QK(r+2) (gated on ACT(r) via the st WAR) so the PE FIFO never
        # parks runnable chain work behind the region gate. On the DVE FIFO
        # the chain copies/normalizes (gated only on PE) come BEFORE
        # masks(r) (gated on ACT(r)) so they never wait behind it.
        # Chain release is smoothed: diagonal blocks cluster at head tails
        # (block widths shrink toward j=15, so several long chains become
        # runnable in the last regions of a head). A pending queue caps the
        # chain work emitted per iteration (~BUDGET PV steps ~= one ACTIVATE
        # of PE time) and spills the excess into the chain-light early
        # regions of the next head.
        BUDGET = 20
        pending = []
        for rr in range(min(2, NR)):
            emit_qk(rr)
        for r in range(NR):
            emit_act(r)
            if r - 1 >= 0:
                pending.extend((h, j) for (h, j, _off) in diags[r - 1])
            steps = 0
            budget = BUDGET if r < NR - 8 else 10 ** 9
            while pending and steps < budget:
                h, s = pending.pop(0)
                emit_chain(h, s)
                steps += s + 1
            emit_masks(r)
            if r + 2 < NR:
                emit_qk(r + 2)
        pending.extend((h, j) for (h, j, _off) in diags[NR - 1])
        for (h, s) in pending:
            emit_chain(h, s)

    nc.compile()
    return nc


def _get_nc():
    global _NC
    if _NC is None:
        _NC = _build_nc()
    return _NC


def _shard_inputs(q, k, v):
    import ml_dtypes
    in_maps = []
    ones = np.ones((SEQ, KV, 1), np.float32)
    for c in range(8):
        b, hg = divmod(c, 4)
        rs = slice(b * SEQ, (b + 1) * SEQ)
        qs = q[rs, hg * 1024:(hg + 1) * 1024]    # [seq, 8*128]
        ks = k[rs, hg * 256:(hg + 1) * 256]      # [seq, 2*128]
        vs = v[rs, hg * 256:(hg + 1) * 256].reshape(SEQ, KV, D)
        vo = np.concatenate([vs, ones], axis=2).reshape(SEQ, KV * (D + 1))
        in_maps.append({
            "qT": np.ascontiguousarray(
                qs.reshape(SEQ, H, D).transpose(2, 1, 0)
            ).astype(ml_dtypes.bfloat16),
            "kT": np.ascontiguousarray(
                ks.reshape(SEQ, KV, D).transpose(2, 1, 0)
            ).astype(ml_dtypes.bfloat16),
            "vones": np.ascontiguousarray(vo).astype(ml_dtypes.bfloat16),
        })
    return in_maps


def _run(q, k, v, **spmd_kwargs):
    from concourse.bass_utils import run_bass_kernel_spmd

    nc = _get_nc()
    bkr = run_bass_kernel_spmd(nc, _shard_inputs(q, k, v),
                               core_ids=list(range(8)), **spmd_kwargs)
    out = np.empty((2 * SEQ, 32 * D), np.float32)
    for c in range(8):
        b, hg = divmod(c, 4)
        out[b * SEQ:(b + 1) * SEQ, hg * 1024:(hg + 1) * 1024] = \
            bkr.results[c]["out"]
    return out, bkr


def kernel(q, k, v, bs=2, seq_len=2048, **_ignored):
    q = np.asarray(q, dtype=np.float32)
    k = np.asarray(k, dtype=np.float32)
    v = np.asarray(v, dtype=np.float32)
    assert int(bs) == 2 and int(seq_len) == SEQ
    assert q.shape == (4096, 4096) and k.shape == (4096, 1024)
    out, _ = _run(q, k, v)
    return out


# revision 22
# speedup vs baseline: 1.0035x; 1.0035x over previous
"""Causal GQA attention on 8 TRN2 NeuronCores.

Problem: q [4096, 4096] = [bs*seq, 32 heads * 128], k/v [4096, 1024] =
[bs*seq, 8 kv heads * 128], causal softmax(q k^T / sqrt(128)) v with GQA
(4 query heads per kv head). f32 in/out.

Sharding: 8 cores = 2 batches x 4 head-groups. Each core owns one batch
and 8 query heads / 2 kv heads -- fully local, no collectives. Q and K are
handed to each core pre-permuted to [head_dim, head, seq] (host-side layout
marshalling in the shard step) so the contraction dim is already on
partitions; V is packed host-side with a fused ones column ([V_j | 1],
bf16) as the PV matmul wants it.

The kernel is organized around the ScalarE exp bottleneck (~139k PSUM
columns of exp per core at ~1 elem/cycle/lane): the S^T columns for all
(head, key-block) pairs form one continuous stream packed into rotating
[128, 1536] f32 PSUM regions (3 banks each, 2 regions = 6 banks), so every
exp is a single wide ACTIVATE (N=1536) instead of many narrow ones --
per-instruction overhead (~400ns) amortizes 12x better. The causal mask is
applied AFTER exp by a DVE multiply of each diagonal 128x128 P^T subtile
with a 0/1 lower-triangle (bf16 2x mode), keeping the QK->exp path free of
VectorE and making softmax denominators exact (masked probs are 0).

Per-core pipeline over regions r (91 per core):
  QK(r+1) on PE (K_j-stationary, <=512-wide bank-aligned chunks)
  || exp(r) on ScalarE (one ACTIVATE, scale folded in)
  || triangle masks(r) + chain normalizes on DVE
  || PV chains whose diagonal block lives in region r-1 on PE:
       acc[q,0:129] = sum_j P^T_j[:, s-subtile] @ [V_j | 1]
     (P^T-stationary accumulation in 2 rotating 1-bank PSUM tiles).

Walrus sync-wait limits (1 slot on DMA descriptors and LDWEIGHTS): all
loads land upfront in fresh buffers, tiny PE warmup matmuls absorb the
DMA semaphores into PE's vector clock (injected just before first use for
late-loaded pieces), and the triangle mask lives in a raw pre-Tile
preamble so it is dependency-free.

No max-subtraction softmax: logits are ~N(0,1) after scale, exp stays in
range; diag-block garbage (upper triangle) is finite and zeroed post-exp.
"""

import numpy as np

P = 128          # partitions / head_dim / key block
SEQ = 2048       # per-core sequence length
H = 8            # query heads per core
KV = 2           # kv heads per core
D = 128          # head dim
NB = SEQ // P    # 16 key blocks (also query subtiles) per head
SCALE = float(D) ** -0.5

REG_W = 1536     # S^T stream region width (3 PSUM banks of f32)
HEAD_W = sum(SEQ - P * j for j in range(NB))          # 17408 cols per head
STREAM_W = H * HEAD_W                                  # 139264 per core
NR = (STREAM_W + REG_W - 1) // REG_W                   # 91 regions

_NC = None


def _block_base(h, j):
    """Stream position of the first column of (head h, key block j)."""
    return h * HEAD_W + SEQ * j - 64 * j * (j - 1)


def _stream_layout():
    """Build-time bookkeeping: region -> QK segments / diag subtiles, and
    (h, j, s) -> (region, offset) for PV chain stationary slices."""
    segs = [[] for _ in range(NR)]    # (h, j, q0, width, region_off)
    diags = [[] for _ in range(NR)]   # (h, j, region_off)
    for h in range(H):
        for j in range(NB):
            w = SEQ - P * j
            base = _block_base(h, j)
            r0, off0 = divmod(base, REG_W)
            diags[r0].append((h, j, off0))
            c = 0
            while c < w:
                r, off = divmod(base + c, REG_W)
                take = min(w - c, REG_W - off, 512 - (off % 512))
                segs[r].append((h, j, P * j + c, take, off))
                c += take
    return segs, diags


def _pt_slice_loc(h, j, s):
    """Region and offset of P^T_j[:, s-th 128-query subtile] for head h."""
    return divmod(_block_base(h, j) + P * (s - j), REG_W)


def _build_nc():
    import concourse.bass as bass
    import concourse.bacc as bacc
    import concourse.mybir as mybir
    import concourse.tile as tile
    from contextlib import ExitStack

    f32 = mybir.dt.float32
    bf16 = mybir.dt.bfloat16
    Exp = mybir.ActivationFunctionType.Exp

    segs, diags = _stream_layout()

    nc = bacc.Bacc()
    qT_ext = nc.declare_dram_parameter("qT", [P, H, SEQ], bf16, isOutput=False)
    kT_ext = nc.declare_dram_parameter("kT", [P, KV, SEQ], bf16, isOutput=False)
    v_ext = nc.declare_dram_parameter("vones", [SEQ, KV * (D + 1)], bf16,
                                      isOutput=False)
    tri_ext = nc.declare_dram_parameter("tri01", [P, P], bf16, isOutput=False)
    o_ext = nc.declare_dram_parameter("out", [SEQ, H * D], f32, isOutput=True)

    vd = v_ext.rearrange("(i p) c -> p i c", p=P)
    od = o_ext.rearrange("(i p) c -> p i c", p=P)

    with ExitStack() as ctx:
        tc = ctx.enter_context(tile.TileContext(nc))
        singles = ctx.enter_context(tc.tile_pool(name="singles", bufs=1))
        pt_pool = ctx.enter_context(tc.tile_pool(name="pt", bufs=20))
        ob_pool = ctx.enter_context(tc.tile_pool(name="ob", bufs=2))
        r_pool = ctx.enter_context(tc.tile_pool(name="r", bufs=8))
        ps_st = ctx.enter_context(tc.tile_pool(name="ps_st", bufs=2, space="PSUM"))
        ps_pv = ctx.enter_context(tc.tile_pool(name="ps_pv", bufs=2, space="PSUM"))

        # ---- upfront loads, each into a fresh buffer on a fresh queue ----
        # Head 0 / kv 0 pieces come first so compute starts early.
        kt = singles.tile([P, KV, SEQ], bf16)        # [d, kv, key]
        qt = singles.tile([P, H, SEQ], bf16)         # [d, head, query]
        vones = singles.tile([P, NB, KV, D + 1], bf16)  # [k, block, kv, d|1]
        tri01 = singles.tile([P, P], bf16)
        nc.gpsimd.dma_start(out=kt[:, 0, 0:256], in_=kT_ext.ap()[:, 0, 0:256])
        nc.gpsimd.dma_start(out=qt[:, 0, 0:1536], in_=qT_ext.ap()[:, 0, 0:1536])
        nc.gpsimd.dma_start(out=tri01, in_=tri_ext.ap())
        nc.gpsimd.dma_start(out=kt[:, 0, 256:1024], in_=kT_ext.ap()[:, 0, 256:1024])
        nc.gpsimd.dma_start(out=qt[:, 0, 1536:], in_=qT_ext.ap()[:, 0, 1536:])
        nc.gpsimd.dma_start(out=kt[:, 0, 1024:], in_=kT_ext.ap()[:, 0, 1024:])
        nc.gpsimd.dma_start(out=vones[:, :, 0, :], in_=vd[:, :, 0:D + 1])
        nc.gpsimd.dma_start(out=qt[:, 1:2, :], in_=qT_ext.ap()[:, 1:2, :])
        nc.gpsimd.dma_start(out=qt[:, 2:3, :], in_=qT_ext.ap()[:, 2:3, :])
        nc.gpsimd.dma_start(out=qt[:, 3:4, :], in_=qT_ext.ap()[:, 3:4, :])
        nc.gpsimd.dma_start(out=kt[:, 1:2, :], in_=kT_ext.ap()[:, 1:2, :])
        nc.gpsimd.dma_start(out=vones[:, :, 1, :], in_=vd[:, :, D + 1:])
        for i in range(4, H):
            nc.gpsimd.dma_start(out=qt[:, i:i + 1, :], in_=qT_ext.ap()[:, i:i + 1, :])

        # Scratch initialized instantly by DVE at t=0: the exp table-load
        # warmup and HAM pre-warm run on it with no DMA dependency.
        scratch = singles.tile([P, P], f32)
        nc.vector.memset(scratch, 0.5)

        # ---- PE warmups: absorb DMA/DVE semaphores into PE's clock so real
        # matmuls never carry a second wait. Outputs unread.
        def warm(ap):
            wm = ps_pv.tile([2, 2], f32, tag="pvacc", name="wm")
            nc.tensor.matmul(wm[:1, :1], lhsT=ap, rhs=ap, start=True, stop=True)

        # HAM pre-warm: ~2.5us of back-to-back dummy matmuls so the PE clock
        # gate opens before the first real QK burst instead of ~20us in.
        # These run while the DMAs land.
        hamwm = ps_pv.tile([P, P], f32, tag="pvacc", name="hamwm")
        scr16 = scratch.bitcast(bf16)
        for _ in range(22):
            nc.tensor.matmul(hamwm, lhsT=scr16[:, 0:P], rhs=scr16[:, 0:P],
                             start=True, stop=True)

        warm(kt[:, 0, 0:1])
        warm(kt[:, 0, 256:257])
        warm(kt[:, 0, 1024:1025])
        warm(qt[:, 0, 0:1])
        warm(qt[:, 0, 1536:1537])
        warm(vones[:, 0, 0, 0:1])
        warm(tri01[:, 0:1])
        # exp table load early, overlapping the remaining DMAs
        actwarm = singles.tile([P, P], bf16)
        nc.scalar.activation(out=actwarm, in_=scratch, func=Exp, scale=SCALE)

        warmed = {("q", 0), ("q", 0.5), ("k", 0), ("k", 0.5), ("v", 0)}

        def warm_for_head(h):
            kvh = h // (H // KV)
            if ("q", h) not in warmed:
                warmed.add(("q", h))
                warm(qt[:, h, 0:1])
            if ("k", kvh) not in warmed:
                warmed.add(("k", kvh))
                warm(kt[:, kvh, 0:1])
            if ("v", kvh) not in warmed:
                warmed.add(("v", kvh))
                warm(vones[:, 0, kvh, 0:1])

        # ---- pipelined region loop ----
        st_tiles = {}
        pt_tiles = {}
        o_sbs = {}

        def emit_qk(r):
            st = ps_st.tile([P, REG_W], f32, name="st")
            st_tiles[r] = st
            for (h, j, q0, w, off) in segs[r]:
                warm_for_head(h)
                kvh = h // (H // KV)
                nc.tensor.matmul(
                    st[:, off:off + w],
                    lhsT=kt[:, kvh, j * P:(j + 1) * P],
                    rhs=qt[:, h, q0:q0 + w],
                    start=True,
                    stop=True,
                )

        def emit_act(r):
            w = min(REG_W, STREAM_W - r * REG_W)
            pt = pt_pool.tile([P, REG_W], bf16, name="pt")
            pt_tiles[r] = pt
            nc.scalar.activation(
                out=pt[:, 0:w], in_=st_tiles.pop(r)[:, 0:w],
                func=Exp, scale=SCALE,
            )

        def emit_masks(r):
            pt = pt_tiles[r]
            for (h, j, off) in diags[r]:
                nc.vector.tensor_mul(
                    out=pt[:, off:off + P],
                    in0=pt[:, off:off + P],
                    in1=tri01,
                )

        def emit_chain(h, s):
            kvh = h // (H // KV)
            if s == 0:
                o_sbs[h] = ob_pool.tile([P, NB, D + 1], f32, name="o_raw")
            acc = ps_pv.tile([P, D + 1], f32, tag="pvacc", name="pvacc")
            for j in range(s + 1):
                rr, off = _pt_slice_loc(h, j, s)
                nc.tensor.matmul(
                    acc,
                    lhsT=pt_tiles[rr][:, off:off + P],
                    rhs=vones[:, j, kvh, :],
                    start=(j == 0),
                    stop=(j == s),
                )
            # One cheap DVE op frees the PSUM acc (pool WAR) quickly; the
            # recip + normalize run from SBUF in batches of 4 subtiles.
            o_raw = o_sbs[h]
            nc.vector.tensor_copy(out=o_raw[:, s, :], in_=acc)
            if s % 4 == 3:
                rcp = r_pool.tile([P, 4], f32, name="rcp")
                nc.vector.reciprocal(rcp, o_raw[:, s - 3:s + 1, D])
                for i in range(4):
                    si = s - 3 + i
                    nc.vector.tensor_scalar_mul(
                        o_raw[:, si, 0:D], o_raw[:, si, 0:D], rcp[:, i:i + 1]
                    )
                nc.sync.dma_start(
                    out=od[:, s - 3:s + 1, h * D:(h + 1) * D],
                    in_=o_raw[:, s - 3:s + 1, 0:D],
                )

        # Iteration r: ACT(r) [needs QK(r), emitted 2 iters ago]; chains for
        # diag region r-1 (runnable: only need ACT(r-1)+mask(r-1)) BEFORE
        # QK(r+2) (gated on ACT(r) via the st WAR) so the PE FIFO never
        # parks runnable chain work behind the region gate. On the DVE FIFO
        # the chain copies/normalizes (gated only on PE) come BEFORE
        # masks(r) (gated on ACT(r)) so they never wait behind it.
        # Chain release is smoothed: diagonal blocks cluster at head tails
        # (block widths shrink toward j=15, so several long chains become
        # runnable in the last regions of a head). A pending queue caps the
        # chain work emitted per iteration (~BUDGET PV steps ~= one ACTIVATE
        # of PE time) and spills the excess into the chain-light early
        # regions of the next head.
        BUDGET = 16
        pending = []
        for rr in range(min(2, NR)):
            emit_qk(rr)
        for r in range(NR):
            emit_act(r)
            if r - 1 >= 0:
                pending.extend((h, j) for (h, j, _off) in diags[r - 1])
            steps = 0
            budget = BUDGET if r < NR - 8 else 10 ** 9
            while pending and steps < budget:
                h, s = pending.pop(0)
                emit_chain(h, s)
                steps += s + 1
            emit_masks(r)
            if r + 2 < NR:
                emit_qk(r + 2)
        pending.extend((h, j) for (h, j, _off) in diags[NR - 1])
        for (h, s) in pending:
            emit_chain(h, s)

    nc.compile()
    return nc


def _get_nc():
    global _NC
    if _NC is None:
        _NC = _build_nc()
    return _NC


def _shard_inputs(q, k, v):
    import ml_dtypes
    in_maps = []
    ones = np.ones((SEQ, KV, 1), np.float32)
    # keep P^T[k, q_local] where q_local >= k
    tri01 = np.triu(np.ones((P, P), np.float32)).astype(ml_dtypes.bfloat16)
    for c in range(8):
        b, hg = divmod(c, 4)
        rs = slice(b * SEQ, (b + 1) * SEQ)
        qs = q[rs, hg * 1024:(hg + 1) * 1024]    # [seq, 8*128]
        ks = k[rs, hg * 256:(hg + 1) * 256]      # [seq, 2*128]
        vs = v[rs, hg * 256:(hg + 1) * 256].reshape(SEQ, KV, D)
        vo = np.concatenate([vs, ones], axis=2).reshape(SEQ, KV * (D + 1))
        in_maps.append({
            "qT": np.ascontiguousarray(
                qs.reshape(SEQ, H, D).transpose(2, 1, 0)
            ).astype(ml_dtypes.bfloat16),
            "kT": np.ascontiguousarray(
                ks.reshape(SEQ, KV, D).transpose(2, 1, 0)
            ).astype(ml_dtypes.bfloat16),
            "vones": np.ascontiguousarray(vo).astype(ml_dtypes.bfloat16),
            "tri01": tri01,
        })
    return in_maps


def _run(q, k, v, **spmd_kwargs):
    from concourse.bass_utils import run_bass_kernel_spmd

    nc = _get_nc()
    bkr = run_bass_kernel_spmd(nc, _shard_inputs(q, k, v),
                               core_ids=list(range(8)), **spmd_kwargs)
    out = np.empty((2 * SEQ, 32 * D), np.float32)
    for c in range(8):
        b, hg = divmod(c, 4)
        out[b * SEQ:(b + 1) * SEQ, hg * 1024:(hg + 1) * 1024] = \
            bkr.results[c]["out"]
    return out, bkr


def kernel(q, k, v, bs=2, seq_len=2048, **_ignored):
    q = np.asarray(q, dtype=np.float32)
    k = np.asarray(k, dtype=np.float32)
    v = np.asarray(v, dtype=np.float32)
    assert int(bs) == 2 and int(seq_len) == SEQ
    assert q.shape == (4096, 4096) and k.shape == (4096, 1024)
    out, _ = _run(q, k, v)
    return out
